# revision 14
# baseline (speedup 1.0000x reference)
"""Trainium2 Bass kernel for nn_Bottleneck_57561151701110 (SAM pairwise
bottleneck block). Data-parallel over batch: 8 images -> 8 NeuronCores.

v2 design (vs baseline): all matmuls fp16 (FWL weight loads, full-rate PE),
conv1/conv2 computed directly in band layout via zero-padded lhsT (no
selector remap stage), x kept resident in SBUF for the residual, consts
packed into one blob DMA, leaky via single ACT Prelu(alpha) ops, softmax
1/Z via DVE reciprocal_approx_fast with e pre-normalized before
aggregation, sam->convo bounce reduced to 2 stores + 16 loads, DMA
triggers spread across sync/scalar/gpsimd queues, and gpsimd cast-DMAs
for the fp16 copy of x.
"""

import os
import sys

for _p in ("/opt/trn_rl_repo", os.path.expanduser("~/.axon_site/_ro/trn_rl_repo")):
    if os.path.isdir(_p) and _p not in sys.path:
        sys.path.insert(0, _p)

from contextlib import ExitStack

import numpy as np

import concourse.bass as bass
import concourse.bacc as bacc
import concourse.tile as tile
from concourse import mybir
from concourse.bass_utils import run_bass_kernel_spmd

dt = mybir.dt
ALU = mybir.AluOpType
ACTF = mybir.ActivationFunctionType

B, CIN, H, W = 8, 256, 56, 56
NPIX = H * W            # 3136
REL, MID, OUT = 32, 256, 256
SHARE = 8
NB = 4                  # row bands
BH = H // NB            # 14 rows per band
Q = BH * W              # 784 band pixels
HBW = Q // 2            # 392 half-band pixels
NEG = 0.01
BN_EPS = 1e-5
OFFS = [(dh, dw) for dh in (-1, 0, 1) for dw in (-1, 0, 1)]

F32, F16 = dt.float32, dt.float16

_CACHE = {}

# blob column layout (fp16)
BL_C1 = 0               # 8 x [128,128]  (kc*4+b)
BL_C2 = 1024
BL_C3 = 2048            # 4 x [128,128]  (kc*2+t)
BL_CO = 2560            # 4 x [128,128]  (t_in*2+o)
BL_W1 = 3072
BL_POST = 3200          # rows 0:8
BL_W2 = 3328
BL_ID = 3456
BL_N = 3584


# ----------------------------------------------------------------- host prep
def _position(h, w):
    loc_w = np.tile(np.linspace(-1.0, 1.0, w, dtype=np.float32)[None, :], (h, 1))
    loc_h = np.tile(np.linspace(-1.0, 1.0, h, dtype=np.float32)[:, None], (1, w))
    return np.stack([loc_w, loc_h], axis=0)  # (2, H, W)


def _host_consts(inp):
    f32 = np.float32
    f16 = np.float16
    inv_a = (inp["bna_g"] / np.sqrt(inp["bna_v"] + BN_EPS)).astype(f32)
    beta_a = (inp["bna_b"] - inp["bna_m"] * inv_a).astype(f32)
    inv_b = (inp["bnb_g"] / np.sqrt(inp["bnb_v"] + BN_EPS)).astype(f32)
    beta_b = (inp["bnb_b"] - inp["bnb_m"] * inv_b).astype(f32)

    w1c = inp["conv1_w"] * inv_a[:REL, None]            # (32, 256)
    b1 = inp["conv1_b"] * inv_a[:REL] + beta_a[:REL]
    w2c = inp["conv2_w"] * inv_a[:REL, None]
    b2 = inp["conv2_b"] * inv_a[:REL]

    blob = np.zeros((128, BL_N), f32)
    # c1z / c2z: zero-padded band lhsT, out col 32b+j = conv channel j
    for kc in range(2):
        for b in range(NB):
            t1 = np.zeros((128, 128), f32)
            t1[:, 32 * b:32 * b + 32] = w1c[:, 128 * kc:128 * (kc + 1)].T
            blob[:, BL_C1 + 128 * (kc * 4 + b):BL_C1 + 128 * (kc * 4 + b + 1)] = t1
            t2 = np.zeros((128, 128), f32)
            t2[:, 32 * b:32 * b + 32] = w2c[:, 128 * kc:128 * (kc + 1)].T
            blob[:, BL_C2 + 128 * (kc * 4 + b):BL_C2 + 128 * (kc * 4 + b + 1)] = t2
    # c3 chunks (kc, t): lhsT[c_local, o_local] = w3[128t+o, 128kc+c]
    w3 = inp["conv3_w"]
    wo = inp["convo_w"]
    for kc in range(2):
        for t in range(2):
            blob[:, BL_C3 + 128 * (kc * 2 + t):BL_C3 + 128 * (kc * 2 + t + 1)] = \
                w3[128 * t:128 * (t + 1), 128 * kc:128 * (kc + 1)].T
    for ti in range(2):
        for o in range(2):
            blob[:, BL_CO + 128 * (ti * 2 + o):BL_CO + 128 * (ti * 2 + o + 1)] = \
                wo[128 * o:128 * (o + 1), 128 * ti:128 * (ti + 1)].T

    # W1' with bnb scale folded; 4-band blockdiag
    w1p = (inp["w1"] * inv_b[:, None]).astype(f32)  # (32, 34)
    w1a, w1b = w1p[:, :REL], w1p[:, REL:]
    lhsT_w1 = np.zeros((128, 128), f32)
    lhsT_pos = np.zeros((8, 128), f32)
    lhsT_w2 = np.zeros((128, 128), f32)
    for b in range(NB):
        lhsT_w1[32 * b:32 * b + 32, 32 * b:32 * b + 32] = w1a.T
        lhsT_pos[2 * b:2 * b + 2, 32 * b:32 * b + 32] = w1b.T
        lhsT_w2[32 * b:32 * b + 32, 32 * b:32 * b + 32] = inp["w2"].T
    blob[:, BL_W1:BL_W1 + 128] = lhsT_w1
    blob[0:8, BL_POST:BL_POST + 128] = lhsT_pos
    blob[:, BL_W2:BL_W2 + 128] = lhsT_w2
    blob[:, BL_ID:BL_ID + 128] = np.eye(128, dtype=f32)

    vecs = np.zeros((128, 8), f32)
    vecs[:, 0] = np.tile(b1, NB)
    vecs[:, 1] = np.tile(b2, NB)
    vecs[:, 2] = inp["conv3_b"][:128]
    vecs[:, 3] = inp["conv3_b"][128:]
    vecs[:, 4] = np.tile(beta_b, NB)
    vecs[:, 5] = np.tile(inp["w2_b"], NB)
    vecs[:, 6] = inp["convo_b"][:128]
    vecs[:, 7] = inp["convo_b"][128:]

    # position branch: posr[2b+c2, Q*k+q] = relu(inv_a*subp + beta_a)
    pos = _position(H, W)
    pc = np.einsum("oc,chw->ohw", inp["convp_w"], pos) + inp["convp_b"][:, None, None]
    pcp = np.pad(pc, ((0, 0), (1, 1), (1, 1)))
    posr = np.zeros((8, 9 * Q), f32)
    for k, (dh, dw) in enumerate(OFFS):
        sub = pc - pcp[:, 1 + dh:1 + dh + H, 1 + dw:1 + dw + W]  # (2,56,56)
        v = np.maximum(inv_a[REL:, None, None] * sub + beta_a[REL:, None, None], 0.0)
        vb = v.reshape(2, NB, BH, W)
        for b in range(NB):
            posr[2 * b:2 * b + 2, Q * k:Q * (k + 1)] = vb[:, b].reshape(2, Q)

    return {
        "cblob": blob.astype(f16),
        "vecs": vecs,
        "posr": posr.astype(f16),
    }


# ------------------------------------------------------------ program build
def _build_program():
    nc = bacc.Bacc("TRN2", target_bir_lowering=False, debug=False,
                   enable_asserts=False, num_devices=8)

    xin = nc.dram_tensor("xin", [CIN, NPIX], F32, kind="ExternalInput").ap()
    cblobd = nc.dram_tensor("cblob", [128, BL_N], F16, kind="ExternalInput").ap()
    vecsd = nc.dram_tensor("vecs", [128, 8], F32, kind="ExternalInput").ap()
    posrd = nc.dram_tensor("posr", [8, 9 * Q], F16, kind="ExternalInput").ap()
    outd = nc.dram_tensor("out", [CIN, NPIX], F32, kind="ExternalOutput").ap()

    # DRAM scratch
    x3d = nc.dram_tensor("x3d", [CIN, 58 * 58], F16).ap()
    samd = nc.dram_tensor("samd", [2 * 128 * 8 * HBW], F16).ap()
    samd_st = samd[:].rearrange("(qp p f) -> qp p f", qp=2, p=128)
    samd_ld = samd[:].rearrange("(qp b c q) -> qp b c q", qp=2, b=NB, c=256)
    x3dv = x3d[:].rearrange("(g s) (r w) -> g s r w", s=SHARE, w=58)

    with tile.TileContext(nc) as tc, ExitStack() as ctx:
        nc_ = tc.nc
        last_w = [None]

        def mm(out, lhsT, rhs, start, stop, key):
            i = nc_.tensor.matmul(out, lhsT, rhs, start=start, stop=stop)
            if key is not None and key == last_w[0]:
                i.ins.ldweights = False
            last_w[0] = key
            return i

        cpool = ctx.enter_context(tc.tile_pool(name="consts", bufs=1))
        xfpool = ctx.enter_context(tc.tile_pool(name="xf", bufs=1))
        xbpool = ctx.enter_context(tc.tile_pool(name="xb", bufs=1))
        fpool = ctx.enter_context(tc.tile_pool(name="feat", bufs=1))
        x3pool = ctx.enter_context(tc.tile_pool(name="x3s", bufs=1))
        xgpool = ctx.enter_context(tc.tile_pool(name="xg", bufs=1))
        epool = ctx.enter_context(tc.tile_pool(name="e", bufs=1))
        fspool = ctx.enter_context(tc.tile_pool(name="fs", bufs=2))
        hppool = ctx.enter_context(tc.tile_pool(name="hp", bufs=2))
        pkpool = ctx.enter_context(tc.tile_pool(name="pk", bufs=3))
        sqpool = ctx.enter_context(tc.tile_pool(name="sq", bufs=2))
        smcpool = ctx.enter_context(tc.tile_pool(name="smc", bufs=8))
        oopool = ctx.enter_context(tc.tile_pool(name="oo", bufs=4))
        o2pool = ctx.enter_context(tc.tile_pool(name="o2", bufs=4))

        # ---- const loads
        cb = cpool.tile([128, BL_N], F16, tag="cb")
        nc.sync.dma_start(cb[:], cblobd[:])
        vec = cpool.tile([128, 8], F32, tag="vec")
        nc.sync.dma_start(vec[:], vecsd[:])
        pos = cpool.tile([8, 9 * Q], F16, tag="pos")
        nc.sync.dma_start(pos[:], posrd[:])

        def c1z(kc, b):
            return cb[:, BL_C1 + 128 * (kc * 4 + b):BL_C1 + 128 * (kc * 4 + b + 1)]

        def c2z(kc, b):
            return cb[:, BL_C2 + 128 * (kc * 4 + b):BL_C2 + 128 * (kc * 4 + b + 1)]

        def c3w(kc, t):
            return cb[:, BL_C3 + 128 * (kc * 2 + t):BL_C3 + 128 * (kc * 2 + t + 1)]

        def cow(ti, o):
            return cb[:, BL_CO + 128 * (ti * 2 + o):BL_CO + 128 * (ti * 2 + o + 1)]

        w1t = cb[:, BL_W1:BL_W1 + 128]
        post = cb[0:8, BL_POST:BL_POST + 128]
        w2t = cb[:, BL_W2:BL_W2 + 128]
        identt = cb[:, BL_ID:BL_ID + 128]

        # ---- x loads (f16 cast chunks now; f32 residual copy loaded late)
        xf = [xfpool.tile([128, NPIX], F32, tag=f"xf{t}", name=f"xf{t}")
              for t in range(2)]
        xb = {}
        for c in range(NB):
            for t in range(2):
                xx = xbpool.tile([128, Q], F16, tag=f"xb{t}{c}")
                nc.gpsimd.dma_start(xx[:], xin[128 * t:128 * (t + 1),
                                               Q * c:Q * (c + 1)])
                xb[(t, c)] = xx

        # ---- zero halo rows of x3d
        zrow = cpool.tile([128, 58], F16, tag="zrow")
        nc_.gpsimd.memset(zrow[:], 0.0)
        for t in range(2):
            tsl = slice(128 * t, 128 * (t + 1))
            nc.sync.dma_start(x3d[tsl, 0:58], zrow[:])
            nc.sync.dma_start(x3d[tsl, 57 * 58:58 * 58], zrow[:])

        # ---- band tiles for feat path
        x1b = fpool.tile([128, Q], F16, tag="x1b")
        x2b = fpool.tile([128, 16, 58], F16, tag="x2b")
        nc_.gpsimd.memset(x2b[:], 0.0)

        # ---- phase A: conv1/conv2 directly in band layout
        pscope1 = ExitStack()
        ppx1 = pscope1.enter_context(tc.tile_pool(name="ppx1", bufs=1, space="PSUM"))
        ppx2 = pscope1.enter_context(tc.tile_pool(name="ppx2", bufs=1, space="PSUM"))
        x1ps = ppx1.tile([128, Q], F32, tag="x1ps")
        x2ps = ppx2.tile([128, 896], F32, tag="x2ps")
        first1 = True
        for kc in range(2):
            for b in range(NB):
                for (o0, n) in ((0, 512), (512, 272)):
                    mm(x1ps[:, o0:o0 + n], c1z(kc, b),
                       xb[(kc, b)][:, o0:o0 + n],
                       start=first1, stop=(kc == 1 and b == NB - 1),
                       key=("c1", kc, b))
                first1 = False
        # x2: band rows 14b-1 .. 14b+14 -> psum cols 0..896, split at the
        # PSUM bank boundary (512). Band order puts full-coverage band 1
        # first so its start=True MMs cover the whole tile; stop=True goes
        # on the last writer of each column region.
        x2mm = []
        for b in range(NB):
            segs = []
            if b > 0:
                segs.append((b - 1, 728, 784, 0, "A"))    # row 14b-1
            segs.append((b, 0, 456, 56, "B1"))
            segs.append((b, 456, 784, 512, "B2"))
            if b < NB - 1:
                segs.append((b + 1, 0, 56, 840, "C"))     # row 14b+14
            x2mm.append(segs)
        border_order = [1, 0, 2, 3]
        for kc in range(2):
            for b in border_order:
                for (cc, s0, s1, d0, kind) in x2mm[b]:
                    st = (kc == 1) and (b == 3 or (b == 2 and kind == "C"))
                    mm(x2ps[:, d0:d0 + (s1 - s0)], c2z(kc, b),
                       xb[(kc, cc)][:, s0:s1],
                       start=(kc == 0 and b == 1), stop=st,
                       key=("c2", kc, b))
        # drains
        nc_.scalar.activation(x1b[:], x1ps[:], ACTF.Identity, bias=vec[:, 0:1])
        nc_.scalar.activation(x2b[0:32, 1:16, 1:57],
                              x2ps[0:32, 56:896]
                              .rearrange("p (r w) -> p r w", w=W),
                              ACTF.Identity, bias=vec[0:32, 1:2])
        nc_.scalar.activation(x2b[32:64, 0:16, 1:57],
                              x2ps[32:64, 0:896]
                              .rearrange("p (r w) -> p r w", w=W),
                              ACTF.Identity, bias=vec[32:64, 1:2])
        nc_.scalar.activation(x2b[64:96, 0:16, 1:57],
                              x2ps[64:96, 0:896]
                              .rearrange("p (r w) -> p r w", w=W),
                              ACTF.Identity, bias=vec[64:96, 1:2])
        nc_.scalar.activation(x2b[96:128, 0:15, 1:57],
                              x2ps[96:128, 0:840]
                              .rearrange("p (r w) -> p r w", w=W),
                              ACTF.Identity, bias=vec[96:128, 1:2])
        pscope1.close()

        # ---- phase B: conv3 -> x3d (fp16, padded 58x58 image)
        x3s = [x3pool.tile([128, 7, 58], F16, tag=f"x3s{i}", name=f"x3s{i}")
               for i in range(2)]
        for i in range(2):
            nc_.gpsimd.memset(x3s[i][:, :, 0:1], 0.0)
            nc_.gpsimd.memset(x3s[i][:, :, 57:58], 0.0)
        pscope2 = ExitStack()
        pp3 = pscope2.enter_context(tc.tile_pool(name="pp3", bufs=2, space="PSUM"))
        for t in range(2):
            for pr in range(4):
                ps3p = []
                for j in range(2):
                    c8 = 2 * pr + j
                    cc, off = c8 // 2, HBW * (c8 % 2)
                    ps3 = pp3.tile([128, HBW], F32, tag="ps3", name=f"ps3_{t}_{c8}")
                    ps3p.append((c8, cc, off, ps3))
                for kc in range(2):
                    for (c8, cc, off, ps3) in ps3p:
                        mm(ps3[:], c3w(kc, t), xb[(kc, cc)][:, off:off + HBW],
                           start=(kc == 0), stop=(kc == 1), key=("c3", kc, t))
                for (c8, cc, off, ps3) in ps3p:
                    xt_ = x3s[c8 % 2]
                    nc_.vector.tensor_scalar(
                        xt_[:, :, 1:57],
                        ps3[:].rearrange("p (r w) -> p r w", w=W),
                        vec[:, 2 + t:3 + t], None, op0=ALU.add)
                    nc.gpsimd.dma_start(
                        x3d[128 * t:128 * (t + 1),
                            58 * (1 + 7 * c8):58 * (1 + 7 * c8 + 7)], xt_[:])

        # ---- xg tiles (A: dw=+-1, B: shifted, dw=0)
        xgA = xgpool.tile([128, SHARE, 16, 58], F16, tag="xgA")
        xgB = xgpool.tile([128, SHARE, 16, 58], F16, tag="xgB")
        for b in range(NB):
            psl = slice(32 * b, 32 * (b + 1))
            nc.gpsimd.dma_start(xgA[psl], x3dv[:, :, 14 * b:14 * b + 16, :])
        nfl = SHARE * 16 * 58
        xgAf = xgA[:].rearrange("p s r w -> p (s r w)")
        xgBf = xgB[:].rearrange("p s r w -> p (s r w)")
        nc_.gpsimd.tensor_copy(xgBf[:, 1:nfl], xgAf[:, 0:nfl - 1])
        for c in range(NB):
            for t in range(2):
                nc.sync.dma_start(xf[t][:, Q * c:Q * (c + 1)],
                                  xin[128 * t:128 * (t + 1), Q * c:Q * (c + 1)])
        xg = {-1: xgA, 0: xgB, 1: xgA}
        xgo = {-1: 0, 0: 2, 1: 2}

        # ---- phase C: per-k logits + exp + Z
        x1bv = x1b[:].rearrange("p (r w) -> p r w", w=W)
        pscope4 = ExitStack()
        ppz = pscope4.enter_context(tc.tile_pool(name="ppz", bufs=1, space="PSUM"))
        pscope3 = ExitStack()
        pph = pscope3.enter_context(tc.tile_pool(name="pph", bufs=2, space="PSUM"))
        zps = ppz.tile([128, Q], F32, tag="zps")
        wsl = [(0, 512), (512, 272)]
        ek = []
        for k, (dh, dw) in enumerate(OFFS):
            fs = fspool.tile([128, BH, W], F16, tag="fs")
            nc_.vector.tensor_tensor(
                fs[:], x1bv,
                x2b[:, 1 + dh:1 + dh + BH, 1 + dw:1 + dw + W],
                ALU.subtract)
            fr = fspool.tile([128, Q], F16, tag="fr")
            nc_.vector.tensor_scalar(fr[:].rearrange("p (r w) -> p r w", w=W),
                                     fs[:], 0.0, None, op0=ALU.max)
            hps = pph.tile([128, Q], F32, tag="hw")
            for (o0, n) in wsl:
                mm(hps[:, o0:o0 + n], w1t, fr[:, o0:o0 + n],
                   start=True, stop=False, key="w1")
            for (o0, n) in wsl:
                mm(hps[:, o0:o0 + n], post,
                   pos[:, Q * k + o0:Q * k + o0 + n],
                   start=False, stop=True, key="post")
            hp = hppool.tile([128, Q], F16, tag="hp")
            if k % 2 == 0:
                nc_.scalar.activation(hp[:], hps[:], ACTF.Relu, bias=vec[:, 4:5])
            else:
                nc_.vector.tensor_scalar(hp[:], hps[:], vec[:, 4:5], 0.0,
                                         op0=ALU.add, op1=ALU.max)
            wps = pph.tile([128, Q], F32, tag="hw")
            for (o0, n) in wsl:
                mm(wps[:, o0:o0 + n], w2t, hp[:, o0:o0 + n],
                   start=True, stop=True, key="w2")
            e = epool.tile([128, Q], F16, tag=f"e{k}")
            nc_.scalar.activation(e[:], wps[:], ACTF.Exp, bias=vec[:, 5:6])
            for (o0, n) in wsl:
                mm(zps[:, o0:o0 + n], identt, e[:, o0:o0 + n],
                   start=(k == 0), stop=(k == 8), key="id")
            ek.append(e)
        pscope3.close()

        # ---- 1/Z and e-prenormalization
        rz = epool.tile([128, Q], F32, tag="rz")
        nc_.vector.reciprocal_approx_fast(rz[:], zps[:])
        pscope4.close()
        pscope2.close()
        rz16 = epool.tile([128, Q], F16, tag="rz16")
        nc_.vector.tensor_copy(rz16[:], rz[:])
        en = []
        for k in range(9):
            e2 = epool.tile([128, Q], F16, tag=f"en{k}")
            nc_.vector.tensor_tensor(e2[:], ek[k][:], rz16[:], ALU.mult)
            en.append(e2)

        # ---- phase E: aggregation rounds + convo per half-band
        pscope5 = ExitStack()
        pps = pscope5.enter_context(tc.tile_pool(name="pps", bufs=1, space="PSUM"))
        ppo = pscope5.enter_context(tc.tile_pool(name="ppo", bufs=4, space="PSUM"))
        NSQ = SHARE // 2
        for qp in range(2):
            sqv = sqpool.tile([128, SHARE, HBW], F16, tag="sqv")
            for sq in range(2):
                sam = pps.tile([128, 2048], F32, tag="sam")
                samv = sam[:].rearrange("p (a j) -> p a j", j=512)[:, :, 0:HBW]
                for k, (dh, dw) in enumerate(OFFS):
                    pk = pkpool.tile([128, NSQ, 7, W], F16, tag="pk")
                    co_ = xgo[dw]
                    r0 = 1 + dh + 7 * qp
                    nc_.vector.tensor_tensor(
                        pk[:],
                        xg[dw][:, NSQ * sq:NSQ * (sq + 1), r0:r0 + 7,
                               co_:co_ + W],
                        en[k][:].rearrange("p (r w) -> p r w", w=W)
                        [:, 7 * qp:7 * qp + 7, :].unsqueeze(1)
                        .broadcast_to((128, NSQ, 7, W)),
                        ALU.mult)
                    pkf = pk[:].rearrange("p a r w -> p (a r w)")
                    for c in range(4):
                        mm(sam[:, 512 * c:512 * c + HBW], identt,
                           pkf[:, HBW * c:HBW * (c + 1)],
                           start=(k == 0), stop=(k == 8), key="id")
                nc_.scalar.activation(sqv[:, NSQ * sq:NSQ * (sq + 1), :], samv,
                                      ACTF.Prelu, alpha=NEG)
            nc.sync.dma_start(samd_st[qp],
                              sqv[:].rearrange("p s q -> p (s q)"))
            smc = {}
            for b in range(NB):
                for t in range(2):
                    s_ = smcpool.tile([128, HBW], F16, tag="smc",
                                      name=f"smc{qp}{b}{t}")
                    nc.sync.dma_start(s_[:],
                                        samd_ld[qp, b, 128 * t:128 * (t + 1), :])
                    smc[(b, t)] = s_
            for o in range(2):
                psos = [ppo.tile([128, HBW], F32, tag="pso", name=f"pso{qp}{o}{b}")
                        for b in range(NB)]
                for ti in range(2):
                    for b in range(NB):
                        mm(psos[b][:], cow(ti, o), smc[(b, ti)][:],
                           start=(ti == 0), stop=(ti == 1), key=("co", ti, o))
                for b in range(NB):
                    po = Q * b + HBW * qp
                    oo = oopool.tile([128, HBW], F32, tag="oo")
                    nc_.scalar.activation(oo[:], psos[b][:], ACTF.Prelu,
                                          bias=vec[:, 6 + o:7 + o], alpha=NEG)
                    o2 = o2pool.tile([128, HBW], F32, tag="o2")
                    nc_.vector.tensor_tensor(o2[:], oo[:],
                                             xf[o][:, po:po + HBW], ALU.add)
                    nc.sync.dma_start(outd[128 * o:128 * (o + 1), po:po + HBW],
                                      o2[:])
        pscope5.close()

    nc.compile()
    return nc


# --------------------------------------------------------------- entrypoint
def _get_program():
    if "nc" not in _CACHE:
        _CACHE["nc"] = _build_program()
    return _CACHE["nc"]


def _run(inputs, trace):
    inputs = {k: np.asarray(v) for k, v in inputs.items()}
    consts = _host_consts(inputs)
    nc = _get_program()
    x = inputs["x"].reshape(B, CIN, NPIX).astype(np.float32)
    in_maps = []
    for b in range(B):
        m = {k: v for k, v in consts.items()}
        m["xin"] = x[b]
        in_maps.append(m)
    res = run_bass_kernel_spmd(nc, in_maps, list(range(B)), trace=trace)
    out = np.stack([res.results[i]["out"] for i in range(B)])
    return out.reshape(B, CIN, H, W).astype(np.float32), res


def kernel(**inputs):
    out, _ = _run(inputs, False)
    return out


def kernel_traced(**inputs):
    """Like kernel() but with NTFF tracing; returns (out, BassKernelResults)."""
    return _run(inputs, True)
